# revision 14
# baseline (speedup 1.0000x reference)
"""Multi-head graph attention (GAT-style) Trainium2 kernel, v2.

Problem: out[b,h,i,o] = softmax_j(mask(leakyrelu_0.2(src[b,h,i] + dst[b,h,j])))
         @ h_prime[b,h,:,:] + bias
with h_prime = h @ w[h], src/dst = tanh(h_prime) @ a_src/a_dst.

Pure data-parallel over the 512-graph batch across 8 NeuronCores (64
graphs/core).  v2 redesign vs the v1 baseline:

  - hpT / hp matmuls row-packed two-at-a-time via tile_position (weights and
    hT replicated on partitions 64-127), halving PE streaming time.
  - S matmul also produces 0.2-scaled src/dst rows (extra a_mats columns) so
    the 0.2x branch of leaky-relu comes out of the PE for free.
  - Attention logits built WITHOUT the additive -250 mask; the adjacency
    mask is applied multiplicatively (0/1) to exp() output on GPSIMD/DVE.
    This kills the identity-matmul mask pass on PE.
  - leakyrelu = Prelu on ACT for heads < K_ACT; for the remaining heads the
    PE computes 0.2*logits into a second PSUM tile and a single DVE
    tensor-max fuses the two branches (1 pass instead of 2).
  - logits matmuls 2x2-way row-packed (K=9 each) via tile_position.
  - numerator per head keeps a ones-column in hp_aug so softmax denominators
    ride the same matmul; the UNNORMALIZED numerator + sums ship to HBM in
    bf16 and the division/bias/transpose happen on the host.
"""

import numpy as np

BS, N, HEADS, DIN, DOUT = 512, 128, 8, 64, 64
NCORES = 8
BSH = BS // NCORES  # graphs per core

K_ACT = 8   # heads 0..K_ACT-1: Prelu on ACT; heads K_ACT..7: DVE 2-op prelu
KA_DVE = 0  # heads 0..KA_DVE-1 mask-mul on DVE; rest on GPSIMD

_cache = {}


def _build_nc():
    import concourse.bass as bass
    import concourse.mybir as mybir
    import concourse.tile as tile

    f32 = mybir.dt.float32
    f16 = mybir.dt.float16
    bf16 = mybir.dt.bfloat16
    AF = mybir.ActivationFunctionType
    HW = DOUT + 1  # 65: per-head numerator cols + row-sum column

    nc = bass.Bass("TRN2", target_bir_lowering=False, debug=False)

    hTr_d = nc.dram_tensor("hTr", [BSH, 128, N], bf16, kind="ExternalInput").ap()
    adjT_d = nc.dram_tensor("adjT", [BSH, N, N], bf16, kind="ExternalInput").ap()
    w_allr_d = nc.dram_tensor("w_allr", [128, HEADS * DOUT], bf16, kind="ExternalInput").ap()
    a_mats_d = nc.dram_tensor("a_mats", [128, 128], bf16, kind="ExternalInput").ap()
    blockind_d = nc.dram_tensor("blockind", [HEADS, HEADS * N], f16, kind="ExternalInput").ap()
    out_d = nc.dram_tensor("out", [BSH, N, HEADS * HW], bf16, kind="ExternalOutput").ap()

    nmax = 8 - K_ACT  # heads handled by the DVE max branch

    with tile.TileContext(nc) as tc:
        with (
            tc.tile_pool(name="consts", bufs=1) as cpool,
            tc.tile_pool(name="inbuf", bufs=2) as inpool,
            tc.tile_pool(name="mid", bufs=2) as midpool,
            tc.tile_pool(name="attn", bufs=2) as attnpool,
            tc.tile_pool(name="outbuf", bufs=2) as outpool,
            # PSUM budget (8 banks): hpT 2 + hp/L02 1 + S 1 + L 2 + num 2
            tc.tile_pool(name="ps_big", bufs=1, space="PSUM") as psbig,
            tc.tile_pool(name="ps_hp", bufs=1, space="PSUM") as pshp,
            tc.tile_pool(name="ps_s", bufs=1, space="PSUM") as pss,
            tc.tile_pool(name="ps_l", bufs=1, space="PSUM") as psl,
            tc.tile_pool(name="ps_num", bufs=1, space="PSUM") as psnum,
        ):
            # ---- constants ----
            w_allr = cpool.tile([128, HEADS * DOUT], bf16, tag="w_allr")
            nc.sync.dma_start(w_allr[:], w_allr_d[:])
            a_mats = cpool.tile([128, 128], bf16, tag="a_mats")
            nc.sync.dma_start(a_mats[:], a_mats_d[:])
            # bi tiles (one per graph parity): rows {1-8,33-40,65-72} hold
            # the constant blockind; rows {0,32} get the per-graph src row
            # and row {64} the 0.2-scaled src row via SBUF DMAs.  (Row 0 of
            # each 32-group = src so the matching ones-row memset in lhsT9
            # lands on a 32-aligned partition, which the verifier requires.)
            bi_ts = []
            for par in range(2):
                bi_t = cpool.tile([73, HEADS * N], f16, tag=f"bi{par}")
                for r0 in (1, 33, 65):
                    nc.sync.dma_start(bi_t[r0 : r0 + 8, :], blockind_d[:])
                bi_ts.append(bi_t)

            for b0 in range(0, BSH, 2):
                # ---- pair inputs ----
                hT_t = inpool.tile([128, 2 * N], bf16, tag="hT")
                nc.sync.dma_start(hT_t[:], hTr_d[b0 : b0 + 2].rearrange("b r n -> r b n"))
                adjT_t = inpool.tile([N, 2 * N], bf16, tag="adjT")
                nc.sync.dma_start(adjT_t[:], adjT_d[b0 : b0 + 2].rearrange("b j i -> j b i"))

                # ---- h_primeT, both graphs, row-packed pairs ----
                # hpT[(q*64+o), col_of[p] + g*128 + n] for head 2p+q.
                # col_of interleaves p-blocks across the two PSUM banks so
                # each concurrent row-packed pair writes different banks.
                col_of = (0, 512, 256, 768)
                hpT_ps = psbig.tile([128, 1024], f32, tag="hpT")
                for p in range(4):
                    ro = 64 * (p % 2)
                    nc.tensor.matmul(
                        hpT_ps[:, col_of[p] : col_of[p] + 256],
                        lhsT=w_allr[ro : ro + 64, p * 128 : (p + 1) * 128],
                        rhs=hT_t[ro : ro + 64, :],
                        start=True,
                        stop=True,
                        tile_position=(ro, 0),
                    )

                tT_t = midpool.tile([128, 1024], bf16, tag="tT")
                nc.scalar.activation(tT_t[:], hpT_ps[:], AF.Tanh)

                # ---- src/dst coefficient rows (and 0.2-scaled copies) ----
                # S[c, g*128+n]: c 0-7 dst, 8-15 src, 16-23 0.2*dst, 24-31 0.2*src
                S_ps = pss.tile([32, 2 * N], f32, tag="S")
                for p in range(4):
                    nc.tensor.matmul(
                        S_ps[:],
                        lhsT=a_mats[:, 32 * p : 32 * (p + 1)],
                        rhs=tT_t[:, col_of[p] : col_of[p] + 256],
                        start=(p == 0),
                        stop=(p == 3),
                    )
                S_sb = midpool.tile([32, 2 * N], f16, tag="S_sb")
                nc.vector.tensor_copy(S_sb[:], S_ps[:])

                for q in range(2):
                    b = b0 + q
                    bi_t = bi_ts[q]
                    qc = slice(q * N, (q + 1) * N)

                    # ---- h_prime natural [n, h*64+o] ----
                    hp_ps = pshp.tile([128, HEADS * DOUT], f32, tag="hp")
                    nc.tensor.matmul(
                        hp_ps[:],
                        lhsT=hT_t[0:64, qc],
                        rhs=w_allr[0:64, :],
                        start=True,
                        stop=True,
                    )
                    hpa_t = midpool.tile([128, HEADS * HW], bf16, tag="hpa")
                    hpa_v = hpa_t[:].rearrange("p (h c) -> p h c", c=HW)
                    nc.gpsimd.memset(hpa_v[:, :, DOUT], 1.0)
                    nc.vector.tensor_copy(
                        hpa_v[:, :, 0:DOUT],
                        hp_ps[:].rearrange("p (h c) -> p h c", c=DOUT),
                    )

                    # ---- stage logits lhsT (ones row + dst rows) ----
                    lhsT9 = midpool.tile([73, N], f16, tag="lhsT9")
                    for r0 in (0, 32, 64):
                        nc.vector.memset(lhsT9[r0 : r0 + 1, :], 1.0)
                    for r0 in (1, 33):  # dst rows
                        nc.sync.dma_start(lhsT9[r0 : r0 + 8, :], S_sb[0:8, qc])
                    nc.sync.dma_start(lhsT9[65:73, :], S_sb[16:24, qc])  # 0.2*dst
                    # src rows -> flattened [1, 1024] rows of bi
                    for r0 in (0, 32):
                        nc.sync.dma_start(
                            bi_t[r0 : r0 + 1, :].rearrange("p (h n) -> p h n", n=N),
                            S_sb[8:16, qc],
                        )
                    nc.sync.dma_start(
                        bi_t[64:65, :].rearrange("p (h n) -> p h n", n=N),
                        S_sb[24:32, qc],
                    )

                    # ---- logits (transposed): L[j, h*128+i] = dst_h[j]+src_h[i]
                    L_ps = psl.tile([128, 1024], f32, tag="L")
                    nc.tensor.matmul(
                        L_ps[:, 0:512], lhsT=lhsT9[0:9, :], rhs=bi_t[0:9, 0:512],
                        start=True, stop=True, tile_position=(0, 0),
                    )
                    nc.tensor.matmul(
                        L_ps[:, 512:1024], lhsT=lhsT9[32:41, :], rhs=bi_t[32:41, 512:1024],
                        start=True, stop=True, tile_position=(32, 0),
                    )
                    # ---- leaky relu: ACT Prelu in place on PSUM for low heads;
                    # DVE 2-op prelu (0.2x to SBUF, then max vs PSUM) for rest.
                    if K_ACT > 0:
                        nc.scalar.activation(
                            L_ps[:, 0 : 128 * K_ACT],
                            L_ps[:, 0 : 128 * K_ACT],
                            AF.Prelu,
                            alpha=0.2,
                        )
                    if nmax > 0:
                        c0 = 128 * K_ACT
                        PR2_t = attnpool.tile([128, 1024 - c0], bf16, tag="PR2")
                        nc.vector.tensor_scalar_mul(PR2_t[:], L_ps[:, c0:1024], 0.2)
                        nc.vector.tensor_max(
                            L_ps[:, c0:1024], L_ps[:, c0:1024], PR2_t[:]
                        )

                    # ---- exp, then multiplicative adjacency mask ----
                    E_t = attnpool.tile([128, 1024], bf16, tag="E")
                    nc.scalar.activation(E_t[:], L_ps[:], AF.Exp)
                    Em_t = attnpool.tile([128, 1024], bf16, tag="Em")
                    adj_q = adjT_t[:, qc]
                    if KA_DVE > 0:
                        nc.vector.tensor_mul(
                            Em_t[:, 0 : 128 * KA_DVE].rearrange("p (h i) -> p h i", i=N),
                            E_t[:, 0 : 128 * KA_DVE].rearrange("p (h i) -> p h i", i=N),
                            adj_q.unsqueeze(1).broadcast_to([N, KA_DVE, N]),
                        )
                    nc.gpsimd.tensor_mul(
                        Em_t[:, 128 * KA_DVE : 1024].rearrange("p (h i) -> p h i", i=N),
                        E_t[:, 128 * KA_DVE : 1024].rearrange("p (h i) -> p h i", i=N),
                        adj_q.unsqueeze(1).broadcast_to([N, 8 - KA_DVE, N]),
                    )

                    # ---- numerator + row sums: num[i, h*65+c], col 64 = s_i
                    numA = psnum.tile([128, 4 * HW], f32, tag="numA")
                    numB = psnum.tile([128, 4 * HW], f32, tag="numB")
                    for h in range(HEADS):
                        dst = numA if h < 4 else numB
                        c0 = (h % 4) * HW
                        nc.tensor.matmul(
                            dst[:, c0 : c0 + HW],
                            lhsT=Em_t[:, h * N : (h + 1) * N],
                            rhs=hpa_t[:, h * HW : (h + 1) * HW],
                            start=True,
                            stop=True,
                        )

                    out_sb = outpool.tile([128, HEADS * HW], bf16, tag="out_sb")
                    nc.vector.tensor_copy(out_sb[:, 0 : 4 * HW], numA[:])
                    nc.vector.tensor_copy(out_sb[:, 4 * HW : 8 * HW], numB[:])
                    nc.sync.dma_start(out_d[b], out_sb[:])

    _split_excess_waits(nc)
    return nc


def _split_excess_waits(nc, cap=1):
    """Walrus codegen accepts at most `cap` sync-wait commands per
    instruction; hoist excess waits onto standalone drains inserted before."""
    import concourse.mybir as mybir

    n_new = 0
    for _bbname, bbw in nc.bb_map.items():
        inner = bbw.bb
        il = list(inner.instructions)
        out, changed = [], False
        for inst in il:
            si = inst.sync_info
            waits = list(si.on_wait) if si and si.on_wait else []
            if len(waits) > cap:
                extra = waits[:-cap]
                for ci in range(0, len(extra), cap):
                    chunk = extra[ci : ci + cap]
                    nop = mybir.InstDrain(
                        name=f"{inst.name}_wsplit{ci}", ins=[], outs=[],
                        bass_is_fusable=False,
                    )
                    nop.engine = inst.engine
                    nop.sync_info = mybir.SyncInfo(on_wait=chunk, on_update=[])
                    nc.register_instruction(nop)
                    out.append(nop)
                    n_new += 1
                si.on_wait = waits[-cap:]
                changed = True
            out.append(inst)
        if changed:
            inner.instructions = out
    return n_new


def _host_prep(h, adj, w, a_src, a_dst):
    import ml_dtypes

    bf = ml_dtypes.bfloat16
    hT = np.ascontiguousarray(h.transpose(0, 2, 1))  # [BS, DIN, N]
    hTr = np.concatenate([hT, hT], axis=1).astype(bf)  # [BS, 128, N]
    adjT = np.ascontiguousarray(adj.transpose(0, 2, 1)).astype(np.float32).astype(bf)
    w_all = np.ascontiguousarray(w.transpose(1, 0, 2).reshape(DIN, HEADS * DOUT))
    w_allr = np.concatenate([w_all, w_all], axis=0).astype(bf)  # [128, 512]
    # a_mats column group p (32 wide, rows (q*64+o) hold head 2p+q):
    #   local col h: a_dst[h]; 8+h: a_src[h]; 16+h: 0.2*a_dst[h]; 24+h: 0.2*a_src[h]
    a_mats = np.zeros((128, 128), np.float32)
    for p in range(4):
        for r in range(2):
            hh = 2 * p + r
            rows = slice(r * 64, (r + 1) * 64)
            a_mats[rows, 32 * p + hh] = a_dst[hh, :, 0]
            a_mats[rows, 32 * p + 8 + hh] = a_src[hh, :, 0]
            a_mats[rows, 32 * p + 16 + hh] = 0.2 * a_dst[hh, :, 0]
            a_mats[rows, 32 * p + 24 + hh] = 0.2 * a_src[hh, :, 0]
    a_mats = a_mats.astype(bf)
    blockind = np.zeros((HEADS, HEADS * N), np.float16)
    for k in range(HEADS):
        blockind[k, k * N : (k + 1) * N] = 1.0
    return hTr, adjT, w_allr, a_mats, blockind


def _make_in_maps(h, adj, w, a_src, a_dst):
    hTr, adjT, w_allr, a_mats, blockind = _host_prep(h, adj, w, a_src, a_dst)
    in_maps = []
    for c in range(NCORES):
        sl = slice(c * BSH, (c + 1) * BSH)
        in_maps.append(
            {
                "hTr": np.ascontiguousarray(hTr[sl]),
                "adjT": np.ascontiguousarray(adjT[sl]),
                "w_allr": w_allr,
                "a_mats": a_mats,
                "blockind": blockind,
            }
        )
    return in_maps


def _gather(results, bias):
    # results[c]["out"]: [BSH, N, HEADS*65] bf16 (num cols 0-63, s col 64)
    full = np.concatenate([results[c]["out"] for c in range(NCORES)], axis=0)
    full = full.astype(np.float32).reshape(BS, N, HEADS, DOUT + 1)
    num = full[..., :DOUT]  # [b, i, h, o]
    s = full[..., DOUT:]  # [b, i, h, 1]
    out = (num / s).transpose(0, 2, 1, 3)  # [b, h, i, o]
    return np.ascontiguousarray(out + bias[None, None, None, :]).astype(np.float32)


def kernel(h, adj, w, a_src, a_dst, bias, _trace=False):
    from concourse.bass_utils import run_bass_kernel_spmd

    h = np.asarray(h, np.float32)
    adj = np.asarray(adj, bool)
    w = np.asarray(w, np.float32)
    a_src = np.asarray(a_src, np.float32)
    a_dst = np.asarray(a_dst, np.float32)
    bias = np.asarray(bias, np.float32)

    if "nc" not in _cache:
        _cache["nc"] = _build_nc()
    nc = _cache["nc"]

    in_maps = _make_in_maps(h, adj, w, a_src, a_dst)
    res = run_bass_kernel_spmd(nc, in_maps, core_ids=list(range(NCORES)), trace=_trace)
    out = _gather(res.results, bias)
    if _trace:
        _cache["last_result"] = res
    return out


# revision 21
# speedup vs baseline: 1.3912x; 1.3912x over previous
"""Multi-head graph attention (GAT-style) Trainium2 kernel, v4.

Problem: out[b,h,i,o] = softmax_j(mask(leakyrelu_0.2(src[b,h,i] + dst[b,h,j])))
         @ h_prime[b,h,:,:] + bias
with h_prime = h @ w[h], src/dst = tanh(h_prime) @ a_src/a_dst.

Pure data-parallel over the 512-graph batch across 8 NeuronCores (64
graphs/core), pipelined across graphs.  Engine assignment per graph:

  PE : h_primeT (row-packed via tile_position), h_prime, src/dst coef rows,
       one K=9 logits matmul, 8 numerator matmuls with a ones-column so
       softmax denominators ride along.
  ACT: tanh (pair-batched), Prelu (in place on PSUM logits), Exp.
  DVE: PSUM->SBUF casts (coef rows, hp_aug, numerator out), 0/1 adjacency
       mask-mul on heads 0-3.
  GPS: mask-mul on heads 4-7, coefficient-gather DMAs.
  DMA: one combined (hT|adjT) load and one output store per PAIR on the
       Sync queue; tiny src/dst coefficient gathers on the GPSIMD queue.

The unnormalized numerator + row sums ship to HBM in bf16; the softmax
division, bias add, and final transpose happen on the host.
"""

import numpy as np

BS, N, HEADS, DIN, DOUT = 512, 128, 8, 64, 64
NCORES = 8
BSH = BS // NCORES  # graphs per core
KA_DVE = 4  # heads 0..KA_DVE-1 mask-mul on DVE; rest on GPSIMD

_cache = {}


def _build_nc():
    import concourse.bass as bass
    import concourse.mybir as mybir
    import concourse.tile as tile

    f32 = mybir.dt.float32
    f16 = mybir.dt.float16
    bf16 = mybir.dt.bfloat16
    AF = mybir.ActivationFunctionType
    HW = DOUT + 1  # 65: per-head numerator cols + row-sum column

    nc = bass.Bass("TRN2", target_bir_lowering=False, debug=False)

    # inp[b] = [hTr | adjT] side by side: cols 0-127 hT (DIN rows doubled),
    # cols 128-255 adjT (0/1).
    inp_d = nc.dram_tensor("inp", [BSH, 128, 2 * N], bf16, kind="ExternalInput").ap()
    w_allr_d = nc.dram_tensor("w_allr", [128, HEADS * DOUT], bf16, kind="ExternalInput").ap()
    a_mats_d = nc.dram_tensor("a_mats", [128, 128], bf16, kind="ExternalInput").ap()
    blockind_d = nc.dram_tensor("blockind", [HEADS, HEADS * N], f16, kind="ExternalInput").ap()
    out_d = nc.dram_tensor("out", [BSH, N, HEADS * HW], bf16, kind="ExternalOutput").ap()

    with tile.TileContext(nc) as tc:
        with (
            tc.tile_pool(name="consts", bufs=1) as cpool,
            tc.tile_pool(name="inbuf", bufs=2) as inpool,
            tc.tile_pool(name="mid", bufs=3) as midpool,
            tc.tile_pool(name="attn", bufs=3) as attnpool,
            tc.tile_pool(name="outbuf", bufs=2) as outpool,
            # PSUM budget (8 banks): hpT 2 + (S|hp shared) 1 + L 2x2 + num 1
            tc.tile_pool(name="ps_big", bufs=1, space="PSUM") as psbig,
            tc.tile_pool(name="ps_hp", bufs=1, space="PSUM") as pshp,
            tc.tile_pool(name="ps_l", bufs=2, space="PSUM") as psl,
            tc.tile_pool(name="ps_num", bufs=1, space="PSUM") as psnum,
        ):
            # ---- constants ----
            w_allr = cpool.tile([128, HEADS * DOUT], bf16, tag="w_allr")
            nc.sync.dma_start(w_allr[:], w_allr_d[:])
            a_mats = cpool.tile([128, 128], bf16, tag="a_mats")
            nc.sync.dma_start(a_mats[:], a_mats_d[:])
            # bi tiles (one per graph parity): rows 1-8 hold the constant
            # blockind; row 0 gets the per-graph flattened src row.
            bi_ts = []
            for par in range(2):
                bi_t = cpool.tile([9, HEADS * N], f16, tag=f"bi{par}")
                nc.sync.dma_start(bi_t[1:9, :], blockind_d[:])
                bi_ts.append(bi_t)

            for b0 in range(0, BSH, 2):
                # ---- pair input: [hT | adjT] for both graphs ----
                in_t = inpool.tile([128, 4 * N], bf16, tag="in")
                nc.sync.dma_start(in_t[:], inp_d[b0 : b0 + 2].rearrange("b r c -> r b c"))

                def hT(g):  # [128, 128] (rows 64-127 duplicate rows 0-63)
                    return in_t[:, g * 2 * N : g * 2 * N + N]

                def adjT(g):  # [128, 128] 0/1
                    return in_t[:, g * 2 * N + N : (g + 1) * 2 * N]

                # ---- h_primeT, both graphs, row-packed pairs ----
                # hpT[(q*64+o), col_of[p]//2 ... ] for head 2p+q; col_of
                # interleaves p-blocks across the two PSUM banks so each
                # concurrent row-packed pair writes different banks.
                col_of = (0, 512, 256, 768)
                hpT_ps = psbig.tile([128, 1024], f32, tag="hpT")
                for p in range(4):
                    ro = 64 * (p % 2)
                    nc.tensor.matmul(
                        hpT_ps[:, col_of[p] : col_of[p] + 256].rearrange(
                            "m (g n) -> m g n", g=2
                        ),
                        lhsT=w_allr[ro : ro + 64, p * 128 : (p + 1) * 128],
                        rhs=in_t[ro : ro + 64, :].rearrange(
                            "k (g c) -> k g c", g=2
                        )[:, :, 0:N],
                        start=True,
                        stop=True,
                        tile_position=(ro, 0),
                    )

                tT_t = midpool.tile([128, 1024], bf16, tag="tT")
                nc.scalar.activation(tT_t[:], hpT_ps[:], AF.Tanh)

                # ---- src/dst coefficient rows ----
                # S[c, g*128+n]: c 0-7 dst, 8-15 src (rows 16-31 unused)
                S_ps = pshp.tile([128, HEADS * DOUT], f32, tag="hp")
                for p in range(4):
                    nc.tensor.matmul(
                        S_ps[0:32, 0 : 2 * N],
                        lhsT=a_mats[:, 32 * p : 32 * (p + 1)],
                        rhs=tT_t[:, col_of[p] : col_of[p] + 256],
                        start=(p == 0),
                        stop=(p == 3),
                    )
                S_sb = midpool.tile([16, 2 * N], f16, tag="S_sb")
                nc.vector.tensor_copy(S_sb[:], S_ps[0:16, 0 : 2 * N])

                # pair logits lhsT: row 0 = ones (written once per ring
                # slot), rows 1-8 = dst coef rows for both graphs.
                lhsT9 = midpool.tile([9, 2 * N], f16, tag="lhsT9")
                # 1 alloc/pair over a 3-deep ring -> init on first 3 pairs
                if b0 < 6:
                    nc.vector.memset(lhsT9[0:1, :], 1.0)
                nc.gpsimd.dma_start(lhsT9[1:9, :], S_sb[0:8, :])

                out_sb = outpool.tile([128, 2 * HEADS * HW], bf16, tag="out_sb")
                for q in range(2):
                    b = b0 + q
                    bi_t = bi_ts[q]
                    qc = slice(q * N, (q + 1) * N)

                    # ---- h_prime natural [n, h*64+o] ----
                    hp_ps = pshp.tile([128, HEADS * DOUT], f32, tag="hp")
                    nc.tensor.matmul(
                        hp_ps[:],
                        lhsT=hT(q)[0:64, :],
                        rhs=w_allr[0:64, :],
                        start=True,
                        stop=True,
                    )
                    hpa_t = midpool.tile([128, HEADS * HW], bf16, tag="hpa")
                    hpa_v = hpa_t[:].rearrange("p (h c) -> p h c", c=HW)
                    # ring slots keep their ones column forever once written;
                    # 2 allocs/pair over a 3-deep ring -> init on first 2 pairs
                    if b0 < 4:
                        nc.gpsimd.memset(hpa_v[:, :, DOUT], 1.0)
                    nc.vector.tensor_copy(
                        hpa_v[:, :, 0:DOUT],
                        hp_ps[:].rearrange("p (h c) -> p h c", c=DOUT),
                    )

                    # src coef rows -> flattened [1, 1024] row 0 of bi
                    nc.gpsimd.dma_start(
                        bi_t[0:1, :].rearrange("p (h n) -> p h n", n=N),
                        S_sb[8:16, qc],
                    )

                    # ---- logits (transposed): L[j, h*128+i] = dst_h[j]+src_h[i]
                    # (two matmuls: a single f32 matmul may not span >1 bank)
                    L_ps = psl.tile([128, 1024], f32, tag="L")
                    for cl in (0, 512):
                        nc.tensor.matmul(
                            L_ps[:, cl : cl + 512],
                            lhsT=lhsT9[:, qc],
                            rhs=bi_t[:, cl : cl + 512],
                            start=True,
                            stop=True,
                        )

                    # ---- leaky relu (in place on PSUM), exp, 0/1 mask ----
                    nc.scalar.activation(L_ps[:], L_ps[:], AF.Prelu, alpha=0.2)
                    E_t = attnpool.tile([128, 1024], bf16, tag="E")
                    nc.scalar.activation(E_t[:], L_ps[:], AF.Exp)
                    Em_t = attnpool.tile([128, 1024], bf16, tag="Em")
                    cm = KA_DVE * N
                    nc.vector.tensor_mul(
                        Em_t[:, 0:cm].rearrange("p (h i) -> p h i", i=N),
                        E_t[:, 0:cm].rearrange("p (h i) -> p h i", i=N),
                        adjT(q).unsqueeze(1).broadcast_to([N, KA_DVE, N]),
                    )
                    nc.gpsimd.tensor_mul(
                        Em_t[:, cm:].rearrange("p (h i) -> p h i", i=N),
                        E_t[:, cm:].rearrange("p (h i) -> p h i", i=N),
                        adjT(q).unsqueeze(1).broadcast_to([N, HEADS - KA_DVE, N]),
                    )

                    # ---- numerator + row sums: num[i, h*65+c], col 64 = s_i
                    for half in range(2):
                        num = psnum.tile([128, 4 * HW], f32, tag="num")
                        for hh in range(4):
                            h = 4 * half + hh
                            nc.tensor.matmul(
                                num[:, hh * HW : (hh + 1) * HW],
                                lhsT=Em_t[:, h * N : (h + 1) * N],
                                rhs=hpa_t[:, h * HW : (h + 1) * HW],
                                start=True,
                                stop=True,
                            )
                        nc.vector.tensor_copy(
                            out_sb[:, (2 * q + half) * 4 * HW : (2 * q + half + 1) * 4 * HW],
                            num[:],
                        )
                    if q == 1:
                        nc.sync.dma_start(
                            out_d[b0 : b0 + 2].rearrange("b n c -> n b c"), out_sb[:]
                        )

    _split_excess_waits(nc)
    return nc


def _split_excess_waits(nc, cap=1):
    """Walrus codegen accepts at most `cap` sync-wait commands per
    instruction; hoist excess waits onto standalone drains inserted before."""
    import concourse.mybir as mybir

    n_new = 0
    for _bbname, bbw in nc.bb_map.items():
        inner = bbw.bb
        il = list(inner.instructions)
        out, changed = [], False
        for inst in il:
            si = inst.sync_info
            waits = list(si.on_wait) if si and si.on_wait else []
            if len(waits) > cap:
                extra = waits[:-cap]
                for ci in range(0, len(extra), cap):
                    chunk = extra[ci : ci + cap]
                    nop = mybir.InstDrain(
                        name=f"{inst.name}_wsplit{ci}", ins=[], outs=[],
                        bass_is_fusable=False,
                    )
                    nop.engine = inst.engine
                    nop.sync_info = mybir.SyncInfo(on_wait=chunk, on_update=[])
                    nc.register_instruction(nop)
                    out.append(nop)
                    n_new += 1
                si.on_wait = waits[-cap:]
                changed = True
            out.append(inst)
        if changed:
            inner.instructions = out
    return n_new


def _host_prep(h, adj, w, a_src, a_dst):
    import ml_dtypes

    bf = ml_dtypes.bfloat16
    hT = np.ascontiguousarray(h.transpose(0, 2, 1))  # [BS, DIN, N]
    inp = np.empty((BS, 128, 2 * N), np.float32)
    inp[:, 0:DIN, 0:N] = hT
    inp[:, DIN:128, 0:N] = hT
    inp[:, :, N:] = adj.transpose(0, 2, 1)  # adjT 0/1
    inp = inp.astype(bf)
    w_all = np.ascontiguousarray(w.transpose(1, 0, 2).reshape(DIN, HEADS * DOUT))
    w_allr = np.concatenate([w_all, w_all], axis=0).astype(bf)  # [128, 512]
    # a_mats column group p (32 wide, rows (q*64+o) hold head 2p+q):
    #   local col h: a_dst[h]; 8+h: a_src[h]
    a_mats = np.zeros((128, 128), np.float32)
    for p in range(4):
        for r in range(2):
            hh = 2 * p + r
            rows = slice(r * 64, (r + 1) * 64)
            a_mats[rows, 32 * p + hh] = a_dst[hh, :, 0]
            a_mats[rows, 32 * p + 8 + hh] = a_src[hh, :, 0]
    a_mats = a_mats.astype(bf)
    blockind = np.zeros((HEADS, HEADS * N), np.float16)
    for k in range(HEADS):
        blockind[k, k * N : (k + 1) * N] = 1.0
    return inp, w_allr, a_mats, blockind


def _make_in_maps(h, adj, w, a_src, a_dst):
    inp, w_allr, a_mats, blockind = _host_prep(h, adj, w, a_src, a_dst)
    in_maps = []
    for c in range(NCORES):
        sl = slice(c * BSH, (c + 1) * BSH)
        in_maps.append(
            {
                "inp": np.ascontiguousarray(inp[sl]),
                "w_allr": w_allr,
                "a_mats": a_mats,
                "blockind": blockind,
            }
        )
    return in_maps


def _gather(results, bias):
    # results[c]["out"]: [BSH, N, HEADS*65] bf16 (num cols 0-63, s col 64)
    full = np.concatenate([results[c]["out"] for c in range(NCORES)], axis=0)
    full = full.astype(np.float32).reshape(BS, N, HEADS, DOUT + 1)
    num = full[..., :DOUT]  # [b, i, h, o]
    s = full[..., DOUT:]  # [b, i, h, 1]
    out = (num / s).transpose(0, 2, 1, 3)  # [b, h, i, o]
    return np.ascontiguousarray(out + bias[None, None, None, :]).astype(np.float32)


def kernel(h, adj, w, a_src, a_dst, bias, _trace=False):
    from concourse.bass_utils import run_bass_kernel_spmd

    h = np.asarray(h, np.float32)
    adj = np.asarray(adj, bool)
    w = np.asarray(w, np.float32)
    a_src = np.asarray(a_src, np.float32)
    a_dst = np.asarray(a_dst, np.float32)
    bias = np.asarray(bias, np.float32)

    if "nc" not in _cache:
        _cache["nc"] = _build_nc()
    nc = _cache["nc"]

    in_maps = _make_in_maps(h, adj, w, a_src, a_dst)
    res = run_bass_kernel_spmd(nc, in_maps, core_ids=list(range(NCORES)), trace=_trace)
    out = _gather(res.results, bias)
    if _trace:
        _cache["last_result"] = res
    return out
